# revision 13
# baseline (speedup 1.0000x reference)
"""Multi-head attention (S=2048, B=2, D=1024, H=16) on 8 Trainium2 cores.

Sharding: tensor-parallel over heads (4 groups of 4 heads) x data-parallel
over batch (2). Core r handles batch r//4, heads [4*(r%4), 4*(r%4)+4).
Each core projects its 256 channels, runs attention for its 4 heads, applies
its slice of the output projection, and a ReduceScatter over each 4-core
batch group sums the partial outputs and leaves each core with a 512-row
slice of the final [2048, 1024] output.

All matmul operands are bf16 (fp32r measures ~2.3x slower per column on this
HW), with fp32 PSUM accumulation. Softmax denominators come free from an
extra ones-column appended to V in the PV matmul. V's bias and the output
bias are folded out algebraically and added on the host.

Schedule: only the K projection and the first query block's Q projection
run before attention. The V projection, the next block's Q projection and
the previous block's output projection are woven into the attention
tk-loop, filling the Tensor engine while the Activation engine (exp) is
the rate limiter. ReduceScatter chunks (bf16) fire per finished block and
overlap the following blocks' compute.
"""
import sys

sys.path.insert(0, "/opt/trn_rl_repo")

import numpy as np
import ml_dtypes
import concourse.bacc as bacc
import concourse.mybir as mybir
from concourse import tile
from concourse.bass_utils import run_bass_kernel_spmd

dt = mybir.dt
AF = mybir.ActivationFunctionType
BF16 = ml_dtypes.bfloat16

S, B, D = 2048, 2, 1024
H, DK = 16, 64
NCORES = 8
HC = 4                 # heads per core
CH = HC * DK           # 256 local channels per core
SCALE = np.float32(1.0 / np.sqrt(DK))
GROUPS = [[0, 1, 2, 3], [4, 5, 6, 7]]

NKD = D // 128         # 8 contraction tiles for projections
NTK = S // 128         # 16 key tiles
TQ = 512               # query block (matmul free dim)
NB = S // TQ           # 4 blocks; block == ReduceScatter chunk


def build_nc():
    f32, bf16 = dt.float32, dt.bfloat16
    nc = bacc.Bacc("TRN2", target_bir_lowering=False, debug=False,
                   num_devices=NCORES)

    xq = nc.dram_tensor("xq_t", [D, S], bf16, kind="ExternalInput").ap()
    xk = nc.dram_tensor("xk_t", [D, S], bf16, kind="ExternalInput").ap()
    xv = nc.dram_tensor("xv_t", [D, S], bf16, kind="ExternalInput").ap()
    wq = nc.dram_tensor("wq_t", [D, CH], bf16, kind="ExternalInput").ap()
    wk = nc.dram_tensor("wk_t", [D, CH], bf16, kind="ExternalInput").ap()
    wv = nc.dram_tensor("wv_t", [D, CH], bf16, kind="ExternalInput").ap()
    wo = nc.dram_tensor("wo_t", [CH, D], bf16, kind="ExternalInput").ap()
    bq = nc.dram_tensor("bq", [2, 128], f32, kind="ExternalInput").ap()
    bk = nc.dram_tensor("bk", [2, 128], f32, kind="ExternalInput").ap()
    ones = nc.dram_tensor("ones", [128, HC], bf16, kind="ExternalInput").ap()
    # Chunk c covers global token rows [c*TQ, (c+1)*TQ); group-rank j
    # receives rows c*TQ + j*128 onward.
    out_ext = nc.dram_tensor("out_rs", [S // 4, D], bf16,
                             kind="ExternalOutput").ap()

    with tile.TileContext(nc) as tc:
        with tc.tile_pool(name="const", bufs=1) as cp, \
             tc.tile_pool(name="stream", bufs=1) as sp, \
             tc.tile_pool(name="psum", bufs=1, space="PSUM") as pp, \
             tc.tile_pool(name="dram", bufs=1, space="DRAM") as dp:

            # ---- resident weights / biases ----
            wq_sb = [cp.tile([128, CH], bf16, tag=f"wq{k}", name=f"wq{k}")
                     for k in range(NKD)]
            wk_sb = [cp.tile([128, CH], bf16, tag=f"wk{k}", name=f"wk{k}")
                     for k in range(NKD)]
            wv_sb = [cp.tile([128, CH], bf16, tag=f"wv{k}", name=f"wv{k}")
                     for k in range(NKD)]
            wo_sb = [cp.tile([128, D], bf16, tag=f"wo{k}", name=f"wo{k}")
                     for k in range(2)]
            bq_sb = [cp.tile([128, 1], f32, tag=f"bq{j}", name=f"bq{j}")
                     for j in range(2)]
            bk_sb = [cp.tile([128, 1], f32, tag=f"bk{j}", name=f"bk{j}")
                     for j in range(2)]
            ones_sb = cp.tile([128, HC], bf16, tag="ones", name="ones_sb")
            # scalar queue: the K path (wk then all of xk), then the rest
            # of the weights. gpsimd carries no loads - software-DGE work
            # there delays partition_broadcast and the collectives.
            for k in range(NKD):
                nc.scalar.dma_start(wk_sb[k][:], wk[k * 128:(k + 1) * 128, :])
            for j in range(2):
                nc.scalar.dma_start(bk_sb[j][:], bk[j].unsqueeze(1))
            xk_sb = {}
            for th in range(NB):
                for k in range(NKD):
                    t_ = sp.tile([128, TQ], bf16, tag="xk", bufs=32,
                                 name=f"xk{th}_{k}")
                    nc.scalar.dma_start(
                        t_[:], xk[k * 128:(k + 1) * 128,
                                  th * TQ:(th + 1) * TQ])
                    xk_sb[(th, k)] = t_
            for j in range(2):
                nc.scalar.dma_start(bq_sb[j][:], bq[j].unsqueeze(1))
            nc.scalar.dma_start(ones_sb[:], ones[:])
            for k in range(NKD):
                nc.scalar.dma_start(wq_sb[k][:], wq[k * 128:(k + 1) * 128, :])
            for k in range(NKD):
                nc.scalar.dma_start(wv_sb[k][:], wv[k * 128:(k + 1) * 128, :])
            for k in range(2):
                nc.scalar.dma_start(wo_sb[k][:], wo[k * 128:(k + 1) * 128, :])

            # sync queue: xv first (needed by the b0 V-weave right at
            # attention start), then xq. All tiles fully resident - ring
            # reuse would head-of-line-block the queue.
            xv_sb = {}
            for tt in range(4):
                for k in range(NKD):
                    t_ = sp.tile([128, TQ], bf16, tag="xv", bufs=32,
                                 name=f"xv{tt}_{k}")
                    nc.sync.dma_start(
                        t_[:], xv[k * 128:(k + 1) * 128,
                                  tt * TQ:(tt + 1) * TQ])
                    xv_sb[(tt, k)] = t_
            xq_sb = {}
            for b in range(NB):
                for k in range(NKD):
                    t_ = sp.tile([128, TQ], bf16, tag="xq", bufs=32,
                                 name=f"xq{b}_{k}")
                    nc.sync.dma_start(
                        t_[:], xq[k * 128:(k + 1) * 128,
                                  b * TQ:(b + 1) * TQ])
                    xq_sb[(b, k)] = t_

            # ---- persistent activations ----
            qc = [cp.tile([128, S], bf16, tag=f"qc{j}", name=f"qc{j}")
                  for j in range(2)]
            kc = [cp.tile([128, S], bf16, tag=f"kc{j}", name=f"kc{j}")
                  for j in range(2)]
            vt = [cp.tile([128, HC * (DK + 1)], bf16, tag=f"vt{t}",
                          name=f"vt{t}") for t in range(NTK)]
            ctx = [cp.tile([128, S], bf16, tag=f"ctx{j}", name=f"ctx{j}")
                   for j in range(2)]
            for t in range(NTK):
                vt_view = vt[t][:].rearrange("p (h c) -> p h c", h=HC)
                nc.vector.tensor_copy(vt_view[:, :, DK:DK + 1],
                                      ones_sb[:].unsqueeze(2))

            # ---- K projection (the only full pre-attention phase) ----
            for th in range(NB):
                for j in range(2):
                    ps = pp.tile([128, TQ], f32, tag="cx", bufs=2,
                                 name=f"kp{th}_{j}")
                    for k in range(NKD):
                        nc.tensor.matmul(
                            ps[:], wk_sb[k][:, j * 128:(j + 1) * 128],
                            xk_sb[(th, k)][:],
                            start=(k == 0), stop=(k == NKD - 1))
                    nc.scalar.activation(
                        kc[j][:, th * TQ:(th + 1) * TQ], ps[:],
                        AF.Identity, bias=bk_sb[j][:, 0:1])

            # ---- Q projection pieces (block b -> one s1-ring slot) ----
            def qproj_matmul(slot, b, m):
                """m-th of 16 matmuls projecting query block b."""
                j, k = m // NKD, m % NKD
                nc.tensor.matmul(
                    slot[:, j * TQ:(j + 1) * TQ],
                    wq_sb[k][:, j * 128:(j + 1) * 128],
                    xq_sb[(b, k)][:],
                    start=(k == 0), stop=(k == NKD - 1))

            def qproj_evict(slot, b):
                for j in range(2):
                    nc.vector.tensor_scalar_add(
                        qc[j][:, b * TQ:(b + 1) * TQ],
                        slot[:, j * TQ:(j + 1) * TQ], bq_sb[j][:, 0:1])

            # ---- V projection piece: one token-tile t, 8-matmul chain ----
            def vproj_tile(t):
                pv = pp.tile([128, 1024], f32, tag="s1", bufs=3,
                             name=f"pv{t}")
                for k in range(NKD):
                    nc.tensor.matmul(
                        pv[:, 0:CH],
                        xv_sb[(t // 4, k)][:, (t % 4) * 128:
                                           (t % 4) * 128 + 128],
                        wv_sb[k][:],
                        start=(k == 0), stop=(k == NKD - 1))
                dst_view = vt[t][:].rearrange("p (h c) -> p h c", h=HC)
                src_view = pv[:, 0:CH].rearrange("p (h c) -> p h c", h=HC)
                nc.vector.tensor_copy(dst_view[:, :, 0:DK], src_view)

            # ---- output projection + store for one 128-row subtile ----
            cc_ins = [dp.tile([TQ, D], bf16, tag=f"ccin{c}",
                              name=f"cc_in{c}") for c in range(NB)]
            cc_outs = [dp.tile([TQ // 4, D], bf16, tag=f"ccout{c}",
                               name=f"cc_out{c}") for c in range(NB)]

            def emit_outproj_subtile(sub):
                chunk = sub // 4
                t0 = sub * 128
                po = pp.tile([128, 1024], f32, tag="s1", bufs=3,
                             name=f"po{sub}")
                for e in range(2):
                    for dv in range(2):
                        nc.tensor.matmul(
                            po[:, e * 512:(e + 1) * 512],
                            ctx[dv][:, t0:t0 + 128],
                            wo_sb[dv][:, e * 512:(e + 1) * 512],
                            start=(dv == 0), stop=(dv == 1))
                osb = sp.tile([128, D], bf16, tag="ot", bufs=8,
                              name=f"ot{sub}")
                nc.vector.tensor_copy(osb[:], po[:])
                r0 = sub * 128 - chunk * TQ
                nc.sync.dma_start(cc_ins[chunk][r0:r0 + 128, :], osb[:])
                if (sub + 1) % 4 == 0:
                    nc.gpsimd.collective_compute(
                        "ReduceScatter", mybir.AluOpType.add,
                        replica_groups=GROUPS,
                        ins=[cc_ins[chunk][:]], outs=[cc_outs[chunk][:]])

            # ---- attention blocks with woven projections ----
            qslot0 = pp.tile([128, 1024], f32, tag="s1", bufs=3,
                             name="qslot0")
            for m in range(16):
                qproj_matmul(qslot0, 0, m)
            qproj_evict(qslot0, 0)
            vproj_tile(0)
            vproj_tile(1)

            for bi in range(NB):
                tq0 = bi * TQ
                for p in range(2):             # head pairs (2p, 2p+1)
                    if p == 1 and bi < NB - 1:
                        qslot = pp.tile([128, 1024], f32, tag="s1", bufs=3,
                                        name=f"qs{bi}")
                    cx = [pp.tile([65, TQ], f32, tag="cx", bufs=2,
                                  name=f"cx{p}_{h}") for h in range(2)]
                    for tk in range(NTK):
                        if bi == 0 and p == 0 and tk < NTK - 2:
                            vproj_tile(tk + 2)
                        if bi > 0 and p == 0 and tk in (4, 7, 10, 13):
                            emit_outproj_subtile((bi - 1) * 4 + (tk - 4) // 3)
                        if p == 1 and bi < NB - 1:
                            qproj_matmul(qslot, bi + 1, tk)
                        s1 = pp.tile([128, 1024], f32, tag="s1", bufs=3,
                                     name=f"s1{tk}")
                        et = sp.tile([128, 1024], bf16, tag="et", bufs=6,
                                     name=f"et{tk}")
                        for h in range(2):      # adjacent -> row-pack overlap
                            r0 = h * 64
                            nc.tensor.matmul(
                                s1[:, h * TQ:(h + 1) * TQ],
                                kc[p][r0:r0 + 64, tk * 128:(tk + 1) * 128],
                                qc[p][r0:r0 + 64, tq0:tq0 + TQ],
                                start=True, stop=True)
                        nc.scalar.activation(et[:], s1[:], AF.Exp)
                        for h in range(2):
                            hl = p * 2 + h
                            nc.tensor.matmul(
                                cx[h][:],
                                vt[tk][:, hl * 65:(hl + 1) * 65],
                                et[:, h * TQ:(h + 1) * TQ],
                                start=(tk == 0), stop=(tk == NTK - 1))
                    if p == 1 and bi < NB - 1:
                        qproj_evict(qslot, bi + 1)
                    cxs = []
                    for h in range(2):
                        c_ = sp.tile([65, TQ], f32, tag="cxs", bufs=4,
                                     name=f"cxs{p}_{h}")
                        nc.vector.tensor_copy(c_[:], cx[h][:])
                        cxs.append(c_)
                    for h in range(2):
                        den = sp.tile([1, TQ], f32, tag="den", bufs=4,
                                      name=f"den{p}_{h}")
                        nc.vector.tensor_copy(den[:], cxs[h][64:65, :])
                        rc = sp.tile([1, TQ], f32, tag="rc", bufs=4,
                                     name=f"rc{p}_{h}")
                        nc.vector.reciprocal_approx_fast(rc[:], den[:])
                        bc = sp.tile([64, TQ], f32, tag="bc", bufs=4,
                                     name=f"bc{p}_{h}")
                        nc.gpsimd.partition_broadcast(bc[:], rc[:])
                        nc.vector.tensor_mul(
                            ctx[p][h * 64:(h + 1) * 64, tq0:tq0 + TQ],
                            cxs[h][0:64, :], bc[:])
            # last block's out-projection
            for s4 in range(4):
                emit_outproj_subtile((NB - 1) * 4 + s4)

            # final stores, force-scheduled at the very end so a store
            # waiting on its ReduceScatter never head-of-line-blocks the
            # sync DMA queue mid-kernel
            with tc.tile_wait_until(10):
                for c in range(NB):
                    nc.sync.dma_start(out_ext[c * 128:(c + 1) * 128, :],
                                      cc_outs[c][:])

    nc.finalize()
    return nc


_NC = None


def _get_nc():
    global _NC
    if _NC is None:
        _NC = build_nc()
    return _NC


def make_in_maps(q, k, v, Wq, bq, Wk, bk, Wv, bv, Wo, bo):
    """Shard + precondition full inputs into per-core input maps."""
    xq_b = [np.ascontiguousarray(q[:, b, :].T).astype(BF16) for b in range(B)]
    xk_b = [np.ascontiguousarray(k[:, b, :].T).astype(BF16) for b in range(B)]
    xv_b = [np.ascontiguousarray(v[:, b, :].T).astype(BF16) for b in range(B)]
    in_maps = []
    for r in range(NCORES):
        b = r // 4
        g = r % 4
        ch = slice(g * CH, (g + 1) * CH)
        in_maps.append({
            "xq_t": xq_b[b], "xk_t": xk_b[b], "xv_t": xv_b[b],
            "wq_t": np.ascontiguousarray((Wq[ch, :] * SCALE).T).astype(BF16),
            "wk_t": np.ascontiguousarray(Wk[ch, :].T).astype(BF16),
            "wv_t": np.ascontiguousarray(Wv[ch, :].T).astype(BF16),
            "wo_t": np.ascontiguousarray(Wo[:, ch].T).astype(BF16),
            "bq": (bq[ch] * SCALE).reshape(2, 128).astype(np.float32),
            "bk": bk[ch].reshape(2, 128).astype(np.float32),
            "ones": np.ones((128, HC), dtype=BF16),
        })
    return in_maps


def assemble(results, Wo, bv, bo):
    """Gather per-core ReduceScatter slices into the full [S, B, D] output."""
    out = np.empty((S, B, D), dtype=np.float32)
    for r in range(NCORES):
        b = r // 4
        j = r % 4
        for c in range(NB):
            g0 = c * TQ + j * 128                # global token rows
            o0 = c * 128                         # rows within out_rs
            out[g0:g0 + 128, b, :] = \
                results[r]["out_rs"][o0:o0 + 128].astype(np.float32)
    out += (bo + Wo @ bv).astype(np.float32)
    return out


def run_sharded(inputs, trace=False):
    nc = _get_nc()
    in_maps = make_in_maps(**inputs)
    res = run_bass_kernel_spmd(nc, in_maps, list(range(NCORES)), trace=trace)
    full = assemble(res.results, np.asarray(inputs["Wo"], dtype=np.float32),
                    np.asarray(inputs["bv"], dtype=np.float32),
                    np.asarray(inputs["bo"], dtype=np.float32))
    return full, res


def kernel(**inputs) -> np.ndarray:
    inputs = {k_: np.asarray(v_, dtype=np.float32)
              for k_, v_ in inputs.items()}
    full, _ = run_sharded(inputs)
    return full


# revision 14
# speedup vs baseline: 1.1200x; 1.1200x over previous
"""Multi-head attention (S=2048, B=2, D=1024, H=16) on 8 Trainium2 cores.

Sharding: tensor-parallel over heads (4 groups of 4 heads) x data-parallel
over batch (2). Core r handles batch r//4, heads [4*(r%4), 4*(r%4)+4).
Each core projects its 256 channels, runs attention for its 4 heads, applies
its slice of the output projection, and a ReduceScatter over each 4-core
batch group sums the partial outputs and leaves each core with a 512-row
slice of the final [2048, 1024] output.

All matmul operands are bf16 (fp32r measures ~2.3x slower per column on this
HW), with fp32 PSUM accumulation. Softmax denominators come free from an
extra ones-column appended to V in the PV matmul. V's bias and the output
bias are folded out algebraically and added on the host.

Schedule: only the K projection and the first query block's Q projection
run before attention. The V projection, the next block's Q projection and
the previous block's output projection are woven into the attention
tk-loop, filling the Tensor engine while the Activation engine (exp) is
the rate limiter. ReduceScatter chunks (bf16) fire per finished block and
overlap the following blocks' compute.
"""
import sys

sys.path.insert(0, "/opt/trn_rl_repo")

import numpy as np
import ml_dtypes
import concourse.bacc as bacc
import concourse.mybir as mybir
from concourse import tile
from concourse.bass_utils import run_bass_kernel_spmd

dt = mybir.dt
AF = mybir.ActivationFunctionType
BF16 = ml_dtypes.bfloat16

S, B, D = 2048, 2, 1024
H, DK = 16, 64
NCORES = 8
HC = 4                 # heads per core
CH = HC * DK           # 256 local channels per core
SCALE = np.float32(1.0 / np.sqrt(DK))
GROUPS = [[0, 1, 2, 3], [4, 5, 6, 7]]

NKD = D // 128         # 8 contraction tiles for projections
NTK = S // 128         # 16 key tiles
TQ = 512               # query block (matmul free dim)
NB = S // TQ           # 4 blocks; block == ReduceScatter chunk


def build_nc():
    f32, bf16 = dt.float32, dt.bfloat16
    nc = bacc.Bacc("TRN2", target_bir_lowering=False, debug=False,
                   num_devices=NCORES)

    xq = nc.dram_tensor("xq_t", [D, S], bf16, kind="ExternalInput").ap()
    xk = nc.dram_tensor("xk_t", [D, S], bf16, kind="ExternalInput").ap()
    xv = nc.dram_tensor("xv_t", [D, S], bf16, kind="ExternalInput").ap()
    wq = nc.dram_tensor("wq_t", [D, CH], bf16, kind="ExternalInput").ap()
    wk = nc.dram_tensor("wk_t", [D, CH], bf16, kind="ExternalInput").ap()
    wv = nc.dram_tensor("wv_t", [D, CH], bf16, kind="ExternalInput").ap()
    wo = nc.dram_tensor("wo_t", [CH, D], bf16, kind="ExternalInput").ap()
    bq = nc.dram_tensor("bq", [2, 128], f32, kind="ExternalInput").ap()
    bk = nc.dram_tensor("bk", [2, 128], f32, kind="ExternalInput").ap()
    ones = nc.dram_tensor("ones", [128, HC], bf16, kind="ExternalInput").ap()
    # Chunk c covers global token rows [c*TQ, (c+1)*TQ); group-rank j
    # receives rows c*TQ + j*128 onward.
    out_ext = nc.dram_tensor("out_rs", [S // 4, D], bf16,
                             kind="ExternalOutput").ap()

    with tile.TileContext(nc) as tc:
        with tc.tile_pool(name="const", bufs=1) as cp, \
             tc.tile_pool(name="stream", bufs=1) as sp, \
             tc.tile_pool(name="psum", bufs=1, space="PSUM") as pp, \
             tc.tile_pool(name="dram", bufs=1, space="DRAM") as dp:

            # ---- resident weights / biases ----
            wq_sb = [cp.tile([128, CH], bf16, tag=f"wq{k}", name=f"wq{k}")
                     for k in range(NKD)]
            wk_sb = [cp.tile([128, CH], bf16, tag=f"wk{k}", name=f"wk{k}")
                     for k in range(NKD)]
            wv_sb = [cp.tile([128, CH], bf16, tag=f"wv{k}", name=f"wv{k}")
                     for k in range(NKD)]
            wo_sb = [cp.tile([128, D], bf16, tag=f"wo{k}", name=f"wo{k}")
                     for k in range(2)]
            bq_sb = [cp.tile([128, 1], f32, tag=f"bq{j}", name=f"bq{j}")
                     for j in range(2)]
            bk_sb = [cp.tile([128, 1], f32, tag=f"bk{j}", name=f"bk{j}")
                     for j in range(2)]
            ones_sb = cp.tile([128, HC], bf16, tag="ones", name="ones_sb")
            # scalar queue: wk, then xk (half at a time, paced so the K
            # projection starts early), with the other weights between the
            # halves. Each dma_start costs ~0.7us of its trigger engine, so
            # all scalar-queue loads land before the exps begin. gpsimd
            # carries no loads - software-DGE work there delays
            # partition_broadcast and the collectives.
            for k in range(NKD):
                nc.scalar.dma_start(wk_sb[k][:], wk[k * 128:(k + 1) * 128, :])
            for j in range(2):
                nc.scalar.dma_start(bk_sb[j][:], bk[j].unsqueeze(1))
            xk_sb = {}
            for hh in range(2):
                for k in range(NKD):
                    t_ = sp.tile([128, 2 * TQ], bf16, tag="xk", bufs=16,
                                 name=f"xk{hh}_{k}")
                    nc.scalar.dma_start(
                        t_[:], xk[k * 128:(k + 1) * 128,
                                  hh * 1024:(hh + 1) * 1024])
                    xk_sb[(hh, k)] = t_
                if hh == 0:
                    for j in range(2):
                        nc.scalar.dma_start(bq_sb[j][:], bq[j].unsqueeze(1))
                    nc.scalar.dma_start(ones_sb[:], ones[:])
                    for k in range(NKD):
                        nc.scalar.dma_start(wq_sb[k][:],
                                            wq[k * 128:(k + 1) * 128, :])
                    for k in range(NKD):
                        nc.scalar.dma_start(wv_sb[k][:],
                                            wv[k * 128:(k + 1) * 128, :])
            for k in range(2):
                nc.scalar.dma_start(wo_sb[k][:], wo[k * 128:(k + 1) * 128, :])

            # sync queue: xq for blocks 0-1 first (Q(b0) gates attention
            # start), then xv (b0's V-weave), then xq for blocks 2-3, then
            # the partial-output stores. All tiles resident - ring reuse
            # would head-of-line-block the queue.
            xq_sb = {}
            for hb in (0, 1):
                if hb == 1:
                    xv_sb = {}
                    for tt in range(4):
                        for k in range(NKD):
                            t_ = sp.tile([128, TQ], bf16, tag="xv", bufs=32,
                                         name=f"xv{tt}_{k}")
                            nc.sync.dma_start(
                                t_[:], xv[k * 128:(k + 1) * 128,
                                          tt * TQ:(tt + 1) * TQ])
                            xv_sb[(tt, k)] = t_
                for k in range(NKD):
                    t_ = sp.tile([128, 2 * TQ], bf16, tag="xq", bufs=16,
                                 name=f"xq{hb}_{k}")
                    nc.sync.dma_start(
                        t_[:], xq[k * 128:(k + 1) * 128,
                                  hb * 1024:(hb + 1) * 1024])
                    xq_sb[(hb, k)] = t_

            # ---- persistent activations ----
            qc = [cp.tile([128, S], bf16, tag=f"qc{j}", name=f"qc{j}")
                  for j in range(2)]
            kc = [cp.tile([128, S], bf16, tag=f"kc{j}", name=f"kc{j}")
                  for j in range(2)]
            vt = [cp.tile([128, HC * (DK + 1)], bf16, tag=f"vt{t}",
                          name=f"vt{t}") for t in range(NTK)]
            ctx = [cp.tile([128, S], bf16, tag=f"ctx{j}", name=f"ctx{j}")
                   for j in range(2)]
            for t in range(NTK):
                vt_view = vt[t][:].rearrange("p (h c) -> p h c", h=HC)
                nc.vector.tensor_copy(vt_view[:, :, DK:DK + 1],
                                      ones_sb[:].unsqueeze(2))

            # ---- K projection (the only full pre-attention phase) ----
            for th in range(NB):
                for j in range(2):
                    ps = pp.tile([128, TQ], f32, tag="cx", bufs=2,
                                 name=f"kp{th}_{j}")
                    for k in range(NKD):
                        nc.tensor.matmul(
                            ps[:], wk_sb[k][:, j * 128:(j + 1) * 128],
                            xk_sb[(th // 2, k)][:, (th % 2) * TQ:
                                                (th % 2) * TQ + TQ],
                            start=(k == 0), stop=(k == NKD - 1))
                    nc.scalar.activation(
                        kc[j][:, th * TQ:(th + 1) * TQ], ps[:],
                        AF.Identity, bias=bk_sb[j][:, 0:1])

            # ---- Q projection pieces (block b -> one s1-ring slot) ----
            def qproj_matmul(slot, b, m):
                """m-th of 16 matmuls projecting query block b."""
                j, k = m // NKD, m % NKD
                nc.tensor.matmul(
                    slot[:, j * TQ:(j + 1) * TQ],
                    wq_sb[k][:, j * 128:(j + 1) * 128],
                    xq_sb[(b // 2, k)][:, (b % 2) * TQ:(b % 2) * TQ + TQ],
                    start=(k == 0), stop=(k == NKD - 1))

            def qproj_evict(slot, b):
                for j in range(2):
                    nc.vector.tensor_scalar_add(
                        qc[j][:, b * TQ:(b + 1) * TQ],
                        slot[:, j * TQ:(j + 1) * TQ], bq_sb[j][:, 0:1])

            # ---- V projection piece: one token-tile t, 8-matmul chain ----
            def vproj_tile(t):
                pv = pp.tile([128, 1024], f32, tag="s1", bufs=3,
                             name=f"pv{t}")
                for k in range(NKD):
                    nc.tensor.matmul(
                        pv[:, 0:CH],
                        xv_sb[(t // 4, k)][:, (t % 4) * 128:
                                           (t % 4) * 128 + 128],
                        wv_sb[k][:],
                        start=(k == 0), stop=(k == NKD - 1))
                dst_view = vt[t][:].rearrange("p (h c) -> p h c", h=HC)
                src_view = pv[:, 0:CH].rearrange("p (h c) -> p h c", h=HC)
                nc.vector.tensor_copy(dst_view[:, :, 0:DK], src_view)

            # ---- output projection + store for one 128-row subtile ----
            cc_ins = [dp.tile([TQ, D], bf16, tag=f"ccin{c}",
                              name=f"cc_in{c}") for c in range(NB)]
            cc_outs = [dp.tile([TQ // 4, D], bf16, tag=f"ccout{c}",
                               name=f"cc_out{c}") for c in range(NB)]

            def emit_outproj_subtile(sub):
                chunk = sub // 4
                t0 = sub * 128
                po = pp.tile([128, 1024], f32, tag="s1", bufs=3,
                             name=f"po{sub}")
                for e in range(2):
                    for dv in range(2):
                        nc.tensor.matmul(
                            po[:, e * 512:(e + 1) * 512],
                            ctx[dv][:, t0:t0 + 128],
                            wo_sb[dv][:, e * 512:(e + 1) * 512],
                            start=(dv == 0), stop=(dv == 1))
                osb = sp.tile([128, D], bf16, tag="ot", bufs=8,
                              name=f"ot{sub}")
                nc.vector.tensor_copy(osb[:], po[:])
                r0 = sub * 128 - chunk * TQ
                nc.sync.dma_start(cc_ins[chunk][r0:r0 + 128, :], osb[:])
                if (sub + 1) % 4 == 0:
                    nc.gpsimd.collective_compute(
                        "ReduceScatter", mybir.AluOpType.add,
                        replica_groups=GROUPS,
                        ins=[cc_ins[chunk][:]], outs=[cc_outs[chunk][:]])

            # ---- attention blocks with woven projections ----
            qslot0 = pp.tile([128, 1024], f32, tag="s1", bufs=3,
                             name="qslot0")
            for m in range(16):
                qproj_matmul(qslot0, 0, m)
            qproj_evict(qslot0, 0)
            vproj_tile(0)
            vproj_tile(1)

            for bi in range(NB):
                tq0 = bi * TQ
                for p in range(2):             # head pairs (2p, 2p+1)
                    if p == 1 and bi < NB - 1:
                        qslot = pp.tile([128, 1024], f32, tag="s1", bufs=3,
                                        name=f"qs{bi}")
                    cx = [pp.tile([65, TQ], f32, tag="cx", bufs=2,
                                  name=f"cx{p}_{h}") for h in range(2)]
                    for tk in range(NTK):
                        if bi == 0 and p == 0 and tk < NTK - 2:
                            vproj_tile(tk + 2)
                        if bi > 0 and p == 0 and tk in (4, 7, 10, 13):
                            emit_outproj_subtile((bi - 1) * 4 + (tk - 4) // 3)
                        if p == 1 and bi < NB - 1:
                            qproj_matmul(qslot, bi + 1, tk)
                        s1 = pp.tile([128, 1024], f32, tag="s1", bufs=3,
                                     name=f"s1{tk}")
                        et = sp.tile([128, 1024], bf16, tag="et", bufs=6,
                                     name=f"et{tk}")
                        for h in range(2):      # adjacent -> row-pack overlap
                            r0 = h * 64
                            nc.tensor.matmul(
                                s1[:, h * TQ:(h + 1) * TQ],
                                kc[p][r0:r0 + 64, tk * 128:(tk + 1) * 128],
                                qc[p][r0:r0 + 64, tq0:tq0 + TQ],
                                start=True, stop=True)
                        nc.scalar.activation(et[:], s1[:], AF.Exp)
                        for h in range(2):
                            hl = p * 2 + h
                            nc.tensor.matmul(
                                cx[h][:],
                                vt[tk][:, hl * 65:(hl + 1) * 65],
                                et[:, h * TQ:(h + 1) * TQ],
                                start=(tk == 0), stop=(tk == NTK - 1))
                    if p == 1 and bi < NB - 1:
                        qproj_evict(qslot, bi + 1)
                    cxs = []
                    for h in range(2):
                        c_ = sp.tile([65, TQ], f32, tag="cxs", bufs=4,
                                     name=f"cxs{p}_{h}")
                        nc.vector.tensor_copy(c_[:], cx[h][:])
                        cxs.append(c_)
                    for h in range(2):
                        den = sp.tile([1, TQ], f32, tag="den", bufs=4,
                                      name=f"den{p}_{h}")
                        nc.vector.tensor_copy(den[:], cxs[h][64:65, :])
                        rc = sp.tile([1, TQ], f32, tag="rc", bufs=4,
                                     name=f"rc{p}_{h}")
                        nc.vector.reciprocal_approx_fast(rc[:], den[:])
                        bc = sp.tile([64, TQ], f32, tag="bc", bufs=4,
                                     name=f"bc{p}_{h}")
                        nc.gpsimd.partition_broadcast(bc[:], rc[:])
                        nc.vector.tensor_mul(
                            ctx[p][h * 64:(h + 1) * 64, tq0:tq0 + TQ],
                            cxs[h][0:64, :], bc[:])
            # last block's out-projection
            for s4 in range(4):
                emit_outproj_subtile((NB - 1) * 4 + s4)

            # final stores, force-scheduled at the very end so a store
            # waiting on its ReduceScatter never head-of-line-blocks the
            # sync DMA queue mid-kernel
            with tc.tile_wait_until(10):
                for c in range(NB):
                    nc.sync.dma_start(out_ext[c * 128:(c + 1) * 128, :],
                                      cc_outs[c][:])

    nc.finalize()
    return nc


_NC = None


def _get_nc():
    global _NC
    if _NC is None:
        _NC = build_nc()
    return _NC


def make_in_maps(q, k, v, Wq, bq, Wk, bk, Wv, bv, Wo, bo):
    """Shard + precondition full inputs into per-core input maps."""
    xq_b = [np.ascontiguousarray(q[:, b, :].T).astype(BF16) for b in range(B)]
    xk_b = [np.ascontiguousarray(k[:, b, :].T).astype(BF16) for b in range(B)]
    xv_b = [np.ascontiguousarray(v[:, b, :].T).astype(BF16) for b in range(B)]
    in_maps = []
    for r in range(NCORES):
        b = r // 4
        g = r % 4
        ch = slice(g * CH, (g + 1) * CH)
        in_maps.append({
            "xq_t": xq_b[b], "xk_t": xk_b[b], "xv_t": xv_b[b],
            "wq_t": np.ascontiguousarray((Wq[ch, :] * SCALE).T).astype(BF16),
            "wk_t": np.ascontiguousarray(Wk[ch, :].T).astype(BF16),
            "wv_t": np.ascontiguousarray(Wv[ch, :].T).astype(BF16),
            "wo_t": np.ascontiguousarray(Wo[:, ch].T).astype(BF16),
            "bq": (bq[ch] * SCALE).reshape(2, 128).astype(np.float32),
            "bk": bk[ch].reshape(2, 128).astype(np.float32),
            "ones": np.ones((128, HC), dtype=BF16),
        })
    return in_maps


def assemble(results, Wo, bv, bo):
    """Gather per-core ReduceScatter slices into the full [S, B, D] output."""
    out = np.empty((S, B, D), dtype=np.float32)
    for r in range(NCORES):
        b = r // 4
        j = r % 4
        for c in range(NB):
            g0 = c * TQ + j * 128                # global token rows
            o0 = c * 128                         # rows within out_rs
            out[g0:g0 + 128, b, :] = \
                results[r]["out_rs"][o0:o0 + 128].astype(np.float32)
    out += (bo + Wo @ bv).astype(np.float32)
    return out


def run_sharded(inputs, trace=False):
    nc = _get_nc()
    in_maps = make_in_maps(**inputs)
    res = run_bass_kernel_spmd(nc, in_maps, list(range(NCORES)), trace=trace)
    full = assemble(res.results, np.asarray(inputs["Wo"], dtype=np.float32),
                    np.asarray(inputs["bv"], dtype=np.float32),
                    np.asarray(inputs["bo"], dtype=np.float32))
    return full, res


def kernel(**inputs) -> np.ndarray:
    inputs = {k_: np.asarray(v_, dtype=np.float32)
              for k_, v_ in inputs.items()}
    full, _ = run_sharded(inputs)
    return full
